# revision 21
# baseline (speedup 1.0000x reference)
"""CondConv2D Trainium2 kernel.

Problem (hardcoded shapes): B=16, C_in=64, H=W=256, E=4, C_out=64, 3x3 conv,
stride=1, dilation=1, padding=1.

Sharding: data-parallel over batch. 8 cores x 2 images each. Expert weights
and routing fc params replicated.

Structure:
  - Host pre-pads each image to 258x258 (zero ring), casts to bf16, stacks
    the two 130-line halves (rows -1..128 / 127..256) into [128, 130, 258]
    (partition = half*64 + c_in). Input DMAs are plain HWDGE chunks with one
    contiguous ~7KB descriptor per partition; the last chunk is small so the
    routing tail after the final DMA is short.
  - Each image is ONE resident SBUF tile. Chunked loads pipeline with the
    pooling, which runs as identity-op + accum_out reduces split into
    half-chunk pieces across DVE (tensor_scalar) and ACT (activation Copy):
    reduce-class ops are 1x-only, so neither engine may fall behind the DMA
    stream, and each block must stay under the psum pipeline slack.
  - Dummy matmuls paced by image-0 chunk arrivals (plus a dense burst on the
    last chunk) keep the PE HAM clock-gate open for the conv start.
  - Routing: pooled sums -> fc via elementwise mul + ones-matmul broadcast
    (bias pre-folded as fc_b/128 so sigmoid reads PSUM directly) -> r[128,4].
    Mixed kernels W_mix = sum_e r_e W_e via a TS(4x)/TT(2x) tree on DVE.
    Image 1's routing is staggered across image 0's conv groups (g=16/20/24)
    so every op lands in its engine queue only after its inputs are ready:
    the Tile scheduler reorders per-engine streams, and an op waiting on a
    slow cross-engine chain head-of-line-blocks all conv drains behind it.
  - Conv: 4 streams (2 halves x 2 pixel streams; px0 = row-pairs 0..31, px1
    = 32..63 of each half) as bf16 matmuls K=64, M=64, N=512 accumulated in
    one 2-bank PSUM tile per group; tile_position packs the streams into the
    4 disjoint 64x64 PE quadrants (runs at ~the 213ns N=512 streaming
    floor). All taps uniform N=512: the padded tile gives im2col for free
    via 2D access patterns.
  - Output: one drain per group [128, 2, 2, 256] f32->bf16 (DVE/ACT 1:1),
    one out-DMA per 16-row stage into a quarter-permuted bf16 layout
    y2[128, 128, 256]; host reassembles + casts to f32. Flushes ride gpsimd
    SWDGE (own queues) so their drain-waits never head-of-line-block the
    input ring; the very last stage flushes in three pieces to shorten the
    kernel tail.
"""
import sys

if "/opt/trn_rl_repo" not in sys.path:
    sys.path.insert(0, "/opt/trn_rl_repo")

import numpy as np

import concourse.bacc as bacc
import concourse.mybir as mybir
import concourse.tile as tile
from concourse.bass_utils import run_bass_kernel_spmd

F32 = mybir.dt.float32
BF16 = mybir.dt.bfloat16
AF = mybir.ActivationFunctionType
ALU = mybir.AluOpType

N_CORES = 8
IMGS_PER_CORE = 2
C_IN = 64
C_OUT = 64
H = 256
W = 256
E = 4
NTAP = 9
WP = W + 2            # padded width
LINES = 130           # lines per half
CHUNKS = [14] * 9 + [4]          # input DMA / reduce chunk lines
N_CHUNKS = len(CHUNKS)
OFFS = [sum(CHUNKS[:i]) for i in range(N_CHUNKS)]
PAIRS = 32            # row-pairs per pixel stream (= quarter image)
STAGE_ROWS = 16       # output rows per quarter per staging tile
PSUM_BUFS = 3

BF16_NP = mybir.dt.np(BF16)


def build_nc():
    nc = bacc.Bacc("TRN2", target_bir_lowering=False, debug=False,
                   num_devices=N_CORES)
    x = nc.dram_tensor("x", [IMGS_PER_CORE, 128, LINES, WP], BF16,
                       kind="ExternalInput")
    wt = nc.dram_tensor("wt", [128, E * NTAP * C_OUT], BF16,
                        kind="ExternalInput")
    fcw = nc.dram_tensor("fcw", [128, E], F32, kind="ExternalInput")
    fcb = nc.dram_tensor("fcb", [128, E], F32, kind="ExternalInput")
    ones = nc.dram_tensor("ones", [128, 128], F32, kind="ExternalInput")
    y = nc.dram_tensor("y", [IMGS_PER_CORE, 128, 128, W], BF16,
                       kind="ExternalOutput")

    gps = STAGE_ROWS // 2              # conv groups per stage block
    S = NTAP * C_OUT

    with tile.TileContext(nc) as tc:
        with (
            tc.tile_pool(name="consts", bufs=1) as consts,
            tc.tile_pool(name="img", bufs=IMGS_PER_CORE) as img_pool,
            tc.tile_pool(name="small", bufs=4) as small,
            tc.tile_pool(name="mix", bufs=2 * IMGS_PER_CORE) as mix_pool,
            tc.tile_pool(name="stage", bufs=2) as stage_pool,
            tc.tile_pool(name="psum", bufs=PSUM_BUFS, space="PSUM") as psum_pool,
        ):
            wtt = consts.tile([128, E * NTAP * C_OUT], BF16)
            fcwt = consts.tile([128, E], F32)
            fcbt = consts.tile([128, E], F32)
            onest = consts.tile([128, 128], F32)
            # consts ride the (idle-early) scalar HWDGE ring
            nc.scalar.dma_start(wtt[:], wt[:])
            nc.scalar.dma_start(fcwt[:], fcw[:])
            nc.scalar.dma_start(fcbt[:], fcb[:])
            nc.scalar.dma_start(onest[:], ones[:])

            xts = [img_pool.tile([128, LINES, WP], BF16, name=f"xt{i}",
                                 tag="xt")
                   for i in range(IMGS_PER_CORE)]
            partials = [small.tile([128, 2 * N_CHUNKS], F32,
                                   name=f"partial{i}", tag="partial")
                        for i in range(IMGS_PER_CORE)]

            # per-engine reduce dump targets (reduces must not rewrite the
            # image tile in place: that would serialize the warm matmuls and
            # conv reads behind the reduce stream)
            dump_v = consts.tile([128, max(CHUNKS) * WP], BF16)
            dump_s = consts.tile([128, max(CHUNKS) * WP], BF16)

            def reduce_one(engine, ap, acc):
                n = ap.shape[-1]
                p0 = ap.base_partition()
                psz = ap.partition_size()
                if engine == "v":
                    nc.vector.tensor_scalar(dump_v[p0:p0 + psz, 0:n], ap, 1.0,
                                            None, op0=ALU.mult, op1=ALU.add,
                                            accum_out=acc)
                else:
                    nc.scalar.activation(dump_s[p0:p0 + psz, 0:n], ap, AF.Copy,
                                         accum_out=acc)

            def reduce_into(engine, ap, acc2):
                """acc2[:, 0:2] = half-sums of ap. Split keeps each engine
                block ~2us, under the psum pipeline slack, so reduces never
                stall the conv drain stream."""
                n = ap.shape[-1]
                h = (n // 2) // WP * WP
                reduce_one(engine, ap[:, 0:h], acc2[:, 0:1])
                reduce_one(engine, ap[:, h:n], acc2[:, 1:2])

            def load_chunk(i, t, warm=False, reduce_now=True):
                """DMA chunk t of image i and accumulate its pooling sums.

                Image 0 splits reduces across DVE/ACT (head latency); image 1
                runs them all on ACT so DVE can own the conv drains.
                """
                xt, partial = xts[i], partials[i]
                r0, ln = OFFS[t], CHUNKS[t]
                nc.sync.dma_start(xt[:, r0:r0 + ln, :],
                                  x[i, :, r0:r0 + ln, :])
                if not reduce_now:
                    return
                do_reduce(i, t)
                if warm:
                    warm_mms(t)

            def do_reduce(i, t):
                xt, partial = xts[i], partials[i]
                r0, ln = OFFS[t], CHUNKS[t]
                # image 1's DVE share is late chunks only: early chunks
                # arrive exactly when mix0 runs on DVE, and the scheduler
                # would hoist their (ready) reduces ahead of it
                eng = ("v" if t % 2 == 0 else "s") if i == 0 else (
                    "v" if t in (5, 7) else "s")
                if t == 0:
                    # bottom lines 0,1 = rows 127,128 already counted in the
                    # top half -> exclude from the bottom sum.
                    top = xt[0:64, 0:ln, :].rearrange("p a b -> p (a b)")
                    bot = xt[64:128, 2:ln, :].rearrange("p a b -> p (a b)")
                    reduce_into(eng, top, partial[0:64, 0:2])
                    reduce_into("s", bot, partial[64:128, 0:2])
                else:
                    fl = xt[:, r0:r0 + ln, :].rearrange("p a b -> p (a b)")
                    reduce_into(eng, fl, partial[:, 2 * t:2 * t + 2])

            def warm_mms(t):
                # dummy matmuls during image 0's load. The burst on the
                # last chunk is dense ~3.5us so the PE HAM un-throttles
                # right before the real conv starts.
                r0 = OFFS[t]
                nburst = 16 if t == N_CHUNKS - 1 else 1
                for _ in range(nburst):
                    wps = psum_pool.tile([128, 2, W], F32, name="warm",
                                         tag="warm", bufs=1)
                    nc.tensor.matmul(
                        wps[0:64].rearrange("p a b -> p (a b)"),
                        wtt[0:64, 0:64], xts[0][0:64, r0:r0 + 2, 1:1 + W],
                        start=True, stop=True, skip_group_check=True)

            # Routing is staged so that every op placed in the DVE queue is
            # ready BEFORE its queue position: a DVE op waiting on a slow
            # cross-engine chain head-of-line-blocks all conv drains queued
            # behind it (the Tile scheduler hoists ready-looking ops).
            def routing_pre(i):
                """pooled sums + fc pre-product (fcbt holds fc_b/128: the
                bias folds into the ones-contraction so sigmoid can read the
                matmul result straight from PSUM)."""
                partial = partials[i]
                pooled = small.tile([128, 1], F32)
                if i == 0:
                    nc.vector.reduce_sum(pooled[:], partial[:],
                                         axis=mybir.AxisListType.X)
                else:
                    # image 1: keep it off the DVE drain stream
                    nc.scalar.activation(dump_s[:, 0:2 * N_CHUNKS], partial[:],
                                         AF.Copy, accum_out=pooled[:])
                tmp4 = small.tile([128, E], F32)
                nc.vector.scalar_tensor_tensor(
                    tmp4[:], fcwt[:], pooled[:, 0:1], fcbt[:],
                    op0=ALU.mult, op1=ALU.add)
                return tmp4

            def routing_mm(tmp4):
                ps4 = psum_pool.tile([128, E], F32, bufs=1)
                nc.tensor.matmul(ps4[:], onest[:], tmp4[:], start=True,
                                 stop=True)
                rt = small.tile([128, E], F32)
                nc.scalar.activation(rt[:], ps4[:], AF.Sigmoid)
                return rt

            def mix_weights(rt):
                # TS (4x bf16) + TT (2x) tree beats a scalar_tensor_tensor
                # chain (1x-rate op measured ~818ns each)
                wmix = mix_pool.tile([128, S], BF16)
                wtmp = mix_pool.tile([128, S], BF16)
                nc.vector.tensor_scalar_mul(wmix[:], wtt[:, 0:S], rt[:, 0:1])
                nc.vector.tensor_scalar_mul(wtmp[:], wtt[:, S:2 * S],
                                            rt[:, 1:2])
                nc.vector.tensor_tensor(wmix[:], wmix[:], wtmp[:], op=ALU.add)
                nc.vector.tensor_scalar_mul(wtmp[:], wtt[:, 2 * S:3 * S],
                                            rt[:, 2:3])
                nc.vector.tensor_tensor(wmix[:], wmix[:], wtmp[:], op=ALU.add)
                nc.vector.tensor_scalar_mul(wtmp[:], wtt[:, 3 * S:4 * S],
                                            rt[:, 3:4])
                nc.vector.tensor_tensor(wmix[:], wmix[:], wtmp[:], op=ALU.add)
                return wmix

            def flush(i, stage, base, lo, hi):
                # gpsimd SWDGE: its own sequencer + DMA queues, so a flush
                # waiting on drains never head-of-line-blocks input chunk
                # dispatches (sync ring) or the ACT reduce stream
                dst = y[i].rearrange("p (s r) w -> p s r w", s=2)
                nc.gpsimd.dma_start(dst[:, :, base + lo:base + hi, :],
                                    stage[:, :, lo:hi, :])

            def conv_group(i, g, wmix, stage, last=False):
                """One group: pairs (g, 32+g) of both halves, 9 taps."""
                xt = xts[i]
                ps = psum_pool.tile([128, 2, 2, W], F32, name="ps", tag="ps")
                outA = ps[:, 0].rearrange("p a b -> p (a b)")
                outB = ps[:, 1].rearrange("p a b -> p (a b)")
                lA = 2 * g
                lB = 64 + 2 * g
                for tap in range(NTAP):
                    kh, kw = divmod(tap, 3)
                    st = tap == 0
                    sp = tap == NTAP - 1
                    lhs_t = wmix[0:64, tap * 64:(tap + 1) * 64]
                    lhs_b = wmix[64:128, tap * 64:(tap + 1) * 64]
                    nc.tensor.matmul(
                        outA[0:64], lhs_t,
                        xt[0:64, lA + kh:lA + kh + 2, kw:kw + W],
                        start=st, stop=sp, tile_position=(0, 0),
                        skip_group_check=True)
                    nc.tensor.matmul(
                        outA[64:128], lhs_b,
                        xt[64:128, lA + kh:lA + kh + 2, kw:kw + W],
                        start=st, stop=sp, tile_position=(64, 64),
                        skip_group_check=True)
                    nc.tensor.matmul(
                        outB[64:128], lhs_t,
                        xt[0:64, lB + kh:lB + kh + 2, kw:kw + W],
                        start=st, stop=sp, tile_position=(0, 64),
                        skip_group_check=True)
                    nc.tensor.matmul(
                        outB[0:64], lhs_b,
                        xt[64:128, lB + kh:lB + kh + 2, kw:kw + W],
                        start=st, stop=sp, tile_position=(64, 0),
                        skip_group_check=True)
                # drain psum -> bf16 staging (DVE:ACT = 2:1)
                r0 = (g % gps) * 2
                dst = stage[:, :, r0:r0 + 2, :]
                if g % 3 == 2:
                    nc.scalar.copy(dst, ps[:])
                else:
                    nc.vector.tensor_copy(dst, ps[:])
                # stage full -> out-DMA (split tail for the very last stage)
                base = (g // gps) * STAGE_ROWS
                if last and (g + 1) % gps == 6:
                    flush(i, stage, base, 0, 12)
                elif last and (g + 1) % gps == 7:
                    flush(i, stage, base, 12, 14)
                elif (g + 1) % gps == 0:
                    if last:
                        flush(i, stage, base, 14, 16)
                    else:
                        flush(i, stage, base, 0, STAGE_ROWS)

            # ---- image 0 load + routing (PE kept warm by dummy matmuls) ----
            for t in range(N_CHUNKS):
                load_chunk(0, t, warm=True)
            wmix0 = mix_weights(routing_mm(routing_pre(0)))

            # ---- conv image 0, interleaving image 1 prefetch ----
            # routing1 is staggered (g=16/20/24) so each piece is ready well
            # before its emission position in every engine queue.
            tmp4_1 = rt1 = wmix1 = None
            stage = None
            for g in range(PAIRS):
                if g % gps == 0:
                    stage = stage_pool.tile([128, 2, STAGE_ROWS, W], BF16)
                if g < N_CHUNKS:
                    load_chunk(1, g, reduce_now=g not in (5, 7))
                conv_group(0, g, wmix0, stage)
                if g == 13:
                    do_reduce(1, 5)
                elif g == 15:
                    do_reduce(1, 7)
                elif g == 20:
                    tmp4_1 = routing_pre(1)
                elif g == 24:
                    rt1 = routing_mm(tmp4_1)
                elif g == 27:
                    wmix1 = mix_weights(rt1)

            # ---- conv image 1 ----
            for g in range(PAIRS):
                if g % gps == 0:
                    stage = stage_pool.tile([128, 2, STAGE_ROWS, W], BF16)
                conv_group(1, g, wmix1, stage, last=g >= 24)
    nc.compile()
    return nc


_NC_CACHE = {}


def _get_nc():
    if "nc" not in _NC_CACHE:
        _NC_CACHE["nc"] = build_nc()
    return _NC_CACHE["nc"]


def _prep_shared(weight, fc_w, fc_b):
    # [E, O, I, KH, KW] -> [I, E, KH, KW, O] -> [64, E*9*64], dup halves
    wt = np.ascontiguousarray(weight.transpose(2, 0, 3, 4, 1)).reshape(
        C_IN, E * NTAP * C_OUT)
    wt = np.concatenate([wt, wt], axis=0).astype(BF16_NP)
    fcw = (np.concatenate([fc_w.T, fc_w.T], axis=0) / (H * W)).astype(
        np.float32)
    fcb = np.tile(fc_b.reshape(1, E) / 128.0, (128, 1)).astype(np.float32)
    ones = np.ones((128, 128), np.float32)
    return wt, fcw, fcb, ones


def _prep_x(inputs):
    # pad to 258x258 zero ring, cast bf16, stack halves -> [B, 128, 130, 258]
    B = inputs.shape[0]
    xp = np.zeros((B, C_IN, H + 2, W + 2), dtype=BF16_NP)
    xp[:, :, 1:H + 1, 1:W + 1] = inputs.astype(BF16_NP)
    xh = np.empty((B, 2, C_IN, LINES, WP), dtype=BF16_NP)
    xh[:, 0] = xp[:, :, 0:LINES, :]
    xh[:, 1] = xp[:, :, H - LINES + 2:H + 2, :]
    return xh.reshape(B, 2 * C_IN, LINES, WP)


def _unpack_y(y2):
    # y2: [IMGS, 128, 128, W] quarter-permuted -> [IMGS, C_OUT, H, W] f32
    out = np.empty((y2.shape[0], C_OUT, H, W), dtype=np.float32)
    out[:, :, 0:64] = y2[:, 0:64, 0:64]
    out[:, :, 64:128] = y2[:, 64:128, 64:128]
    out[:, :, 128:192] = y2[:, 64:128, 0:64]
    out[:, :, 192:256] = y2[:, 0:64, 64:128]
    return out


def kernel(inputs, weight, fc_w, fc_b, stride=1, dilation=1, padding=1,
           _trace=False):
    assert int(stride) == 1 and int(dilation) == 1 and int(padding) == 1
    inputs = np.asarray(inputs, dtype=np.float32)
    B = inputs.shape[0]
    assert B == N_CORES * IMGS_PER_CORE
    wt, fcw, fcb, ones = _prep_shared(np.asarray(weight), np.asarray(fc_w),
                                      np.asarray(fc_b))
    xh = _prep_x(inputs)
    nc = _get_nc()
    in_maps = []
    for c in range(N_CORES):
        in_maps.append({
            "x": np.ascontiguousarray(xh[2 * c:2 * c + 2]),
            "wt": wt, "fcw": fcw, "fcb": fcb, "ones": ones,
        })
    res = run_bass_kernel_spmd(nc, in_maps, core_ids=list(range(N_CORES)),
                               trace=_trace)
    out = np.concatenate(
        [_unpack_y(res.results[c]["y"]) for c in range(N_CORES)], axis=0)
    if _trace:
        return out, res
    return out
